# revision 34
# baseline (speedup 1.0000x reference)
"""Multi-head attention (S=2048, B=2, D=1024, H=16) on 8 TRN2 NeuronCores.

Sharding: batch*heads across cores — core c owns heads {2c, 2c+1} for both
batches (4 (head, batch) pairs per core, d_k=64 each -> a 128-row slice of
every projection). The output projection is row-parallel after an AllToAll
that redistributes per-head context to per-sequence-block context.

Schedule (single NEFF per core):
  - All DMAs issue from the SP queue (gpsimd/SWDGE DMAs are broken in this
    stack; ACT-issued DMAs steal exp time). x^T streams in column halves,
    kt-outer per (tensor, half) so x tiles are used once and freed (keeps
    the SP queue flowing); wo preloads via the scalar queue.
  - QKV projections kt-outer over column halves: 2 live [128,1024] PSUM
    accumulators (4 banks) per half; per-batch qT/kT/vT SBUF tiles so
    attention chunk deps don't serialize on the other batch's writes.
  - V_aug blocks ([128j, 64d|ones]) via DMA-transpose on SP per batch.
  - Flash-style attention per (head, batch) in S^T orientation:
      S^T tile = K^T_blk.T @ Q^T  (K=64), expS = ACT Exp(0.125 S^T) -> bf16,
      ctx^T[d|Z, i] += V_aug.T @ expS  (M=65 carries the Z row).
    ACT (exp) and PE trade the bottleneck at ~1.4us per j-tile.
  - Normalization per chunk, overlapped under later chunks: ctx + Z rows to
    SBUF (DVE), Z bounced through DRAM with a partition-broadcast read,
    reciprocal_approx_fast on the [128,ICH] broadcast, one multiply, then
    staged to a2a_in. Batch 0's AllToAll runs under batch 1's attention;
    batch 1's AllToAll + row-parallel O-proj form the tail.
"""

import numpy as np
import ml_dtypes

import concourse.bass as bass
import concourse.mybir as mybir
import concourse.tile as tile
from concourse import bacc
from concourse.bass_utils import run_bass_kernel_spmd

S = 2048
B = 2
D = 1024
H = 16
DK = 64
N_CORES = 8
SCALE = 1.0 / np.sqrt(DK)

F32 = mybir.dt.float32
BF16 = mybir.dt.bfloat16

SB = S * B                      # 4096 total cols (i = b*S + s)
ROWS_PER_CORE = SB // N_CORES   # 512 output rows per core
SEQ_PER_CORE = S // N_CORES     # 256

_cached = {}


def build_program():
    if "nc" in _cached:
        return _cached["nc"]
    nc = bacc.Bacc("TRN2", target_bir_lowering=False, debug=False,
                   num_devices=N_CORES)

    xT = {t: nc.dram_tensor(f"x{t}T", [D, SB], BF16, kind="ExternalInput")
          for t in "qkv"}
    wT = {t: nc.dram_tensor(f"w{t}T", [D, 128], BF16, kind="ExternalInput")
          for t in "qkv"}
    bvec = {t: nc.dram_tensor(f"b{t}", [128, 1], F32, kind="ExternalInput")
            for t in "qkv"}
    woT = nc.dram_tensor("woT", [D, D], BF16, kind="ExternalInput")
    bo_bc = nc.dram_tensor("bo_bc", [128, D], F32, kind="ExternalInput")
    out_d = nc.dram_tensor("out", [ROWS_PER_CORE, D], F32, kind="ExternalOutput")

    a2a_in = [nc.dram_tensor(f"a2a_in{b}", [N_CORES * 128, SEQ_PER_CORE], BF16)
              for b in range(B)]
    a2a_out = [nc.dram_tensor(f"a2a_out{b}", [N_CORES * 128, SEQ_PER_CORE], BF16)
               for b in range(B)]
    zscr = [nc.dram_tensor(f"zscr{b}", [2, S], F32) for b in range(B)]

    with tile.TileContext(nc) as tc:
        _emit(nc, tc, xT, wT, bvec, woT, bo_bc, out_d, a2a_in, a2a_out, zscr)
    nc.compile()
    _cached["nc"] = nc
    return nc


def _emit(nc, tc, xT, wT, bvec, woT, bo_bc, out_d, a2a_in, a2a_out, zscr):
    from contextlib import ExitStack

    ICH = 1024          # i-chunk width
    NMM = 512           # max free dim per matmul into one PSUM bank
    JT = S // 128       # 16 j-tiles per (head, batch) pair
    VA = 128            # V_aug block stride
    NCH = SB // ICH     # 4 chunks total (2 per batch)

    with ExitStack() as top:
        const = top.enter_context(tc.tile_pool(name="const", bufs=1))
        w_sb = const.tile([128, 3 * 8 * 128], BF16)
        bias_sb = const.tile([128, 3], F32)
        # per-batch halves so attention chunk deps don't serialize on the
        # other batch's projection writes (dep tracking is per-tile)
        qT_sb = [const.tile([128, S], BF16, name=f"qT{h}") for h in range(2)]
        kT_sb = [const.tile([128, S], BF16, name=f"kT{h}") for h in range(2)]
        vT_sb = [const.tile([128, S], BF16, name=f"vT{h}") for h in range(2)]
        vaug_sb = const.tile([128, 4 * JT * VA], BF16)
        ones_sb = const.tile([1, 128], BF16)
        bo_sb = const.tile([128, D], F32)
        wo_sb = const.tile([128, 2 * N_CORES * NMM], BF16)
        osrc_sb = const.tile([128, B * N_CORES * SEQ_PER_CORE], BF16)

        nc.vector.memset(vaug_sb[:], 1.0)
        nc.vector.memset(ones_sb[:], 1.0)

        xpool = top.enter_context(tc.tile_pool(name="xstream", bufs=6))
        epool = top.enter_context(tc.tile_pool(name="expS", bufs=8))
        zpool = top.enter_context(tc.tile_pool(name="zrow", bufs=3))
        zbpool = top.enter_context(tc.tile_pool(name="zbr", bufs=5))
        cupool = top.enter_context(tc.tile_pool(name="ctxu", bufs=3))
        cnpool = top.enter_context(tc.tile_pool(name="ctxn", bufs=3))
        outpool = top.enter_context(tc.tile_pool(name="oout", bufs=3))
        pools = {}

        proj_targets = {"q": qT_sb, "k": kT_sb, "v": vT_sb}

        # All DMAs on SP (the baseline-proven queue). x^T streams in column
        # halves (half = batch): k,v,q of batch 0 first so batch-0 attention
        # can start while batch-1 x still streams. wo on the scalar queue
        # (ACT is idle until the exps begin).
        def emit_loads(half):
            for t in "kvq":
                ti = "qkv".index(t)
                if half == 0:
                    for kt in range(8):
                        nc.sync.dma_start(
                            w_sb[:, (ti * 8 + kt) * 128:(ti * 8 + kt + 1) * 128],
                            wT[t].ap()[kt * 128:(kt + 1) * 128, :])
                    nc.sync.dma_start(bias_sb[:, ti:ti + 1], bvec[t].ap())
                for kt in range(8):
                    xt = xpool.tile([128, SB // 2], BF16, tag="xs",
                                    name=f"xs_{t}{half}_{kt}")
                    nc.sync.dma_start(
                        xt[:], xT[t].ap()[kt * 128:(kt + 1) * 128,
                                          half * (SB // 2):(half + 1) * (SB // 2)])
                    xtiles[(t, half, kt)] = xt

        xtiles = {}
        emit_loads(0)
        emit_loads(1)
        nc.sync.dma_start(bo_sb[:], bo_bc.ap())
        for ce in range(D // NMM):
            for s in range(N_CORES):
                nc.scalar.dma_start(
                    wo_sb[:, (ce * N_CORES + s) * NMM:
                          (ce * N_CORES + s + 1) * NMM],
                    woT.ap()[s * 128:(s + 1) * 128, ce * NMM:(ce + 1) * NMM])

        def emit_proj_half(t, half):
            # kt-outer over one column half; both chunk psums live (4 banks)
            ti = "qkv".index(t)
            pss = [pools["pp"].tile([128, ICH], F32, tag="pp",
                                    name=f"pp_{t}{half}_{c}") for c in range(2)]
            for kt in range(8):
                xt = xtiles[(t, half, kt)]
                for c in range(2):
                    for nn in range(2):
                        nc.tensor.matmul(
                            pss[c][:, nn * NMM:(nn + 1) * NMM],
                            w_sb[:, (ti * 8 + kt) * 128:(ti * 8 + kt + 1) * 128],
                            xt[:, c * ICH + nn * NMM:c * ICH + (nn + 1) * NMM],
                            start=(kt == 0), stop=(kt == 7))
            for c in range(2):
                nc.vector.tensor_scalar_add(
                    proj_targets[t][half][:, c * ICH:(c + 1) * ICH],
                    pss[c][:], bias_sb[:, ti:ti + 1])

        def emit_transposes(b):
            for jt in range(JT):
                for hh in range(2):
                    p = (hh * 2 + b)
                    col = (p * JT + jt) * VA
                    nc.sync.dma_start_transpose(
                        vaug_sb[:, col:col + 64],
                        vT_sb[b][hh * 64:hh * 64 + 64,
                                 jt * 128:(jt + 1) * 128])

        def emit_attention_chunk(ch):
            b, ch2 = ch // 2, ch % 2
            ioff = ch2 * ICH
            cps = [pools["cp"].tile([65, ICH], F32, tag="cp",
                                    name=f"cp{ch}_{i}") for i in range(2)]
            for jt in range(JT):
                for hh in range(2):
                    p = hh * 2 + b
                    sps = pools["sp"].tile([128, ICH], F32, tag="sp",
                                           name=f"sp{ch}_{jt}_{hh}")
                    for nn in range(2):
                        nc.tensor.matmul(
                            sps[:, nn * NMM:(nn + 1) * NMM],
                            kT_sb[b][hh * 64:hh * 64 + 64,
                                     jt * 128:(jt + 1) * 128],
                            qT_sb[b][hh * 64:hh * 64 + 64,
                                     ioff + nn * NMM:ioff + (nn + 1) * NMM],
                            start=True, stop=True)
                    es = epool.tile([128, ICH], BF16, tag="es",
                                    name=f"es{ch}_{jt}_{hh}")
                    nc.scalar.activation(
                        es[:], sps[:], mybir.ActivationFunctionType.Exp,
                        scale=float(SCALE))
                    col = (p * JT + jt) * VA
                    for nn in range(2):
                        nc.tensor.matmul(
                            cps[hh][:, nn * NMM:(nn + 1) * NMM],
                            vaug_sb[:, col:col + 65],
                            es[:, nn * NMM:(nn + 1) * NMM],
                            start=(jt == 0), stop=(jt == JT - 1))
            return cps

        def emit_norm_stage(ch, cps):
            # Per-chunk: Z -> recip -> DRAM broadcast bounce -> normalize the
            # two 64-row head groups -> stage into a2a_in. ctx rows for head
            # pair hh live on PSUM partitions 0..63 of cps[hh]; keep all DVE
            # ops partition-aligned (0..63) and let DMA do the row placement.
            b, ch2 = ch // 2, ch % 2
            # Per chunk: ctx + Z rows staged to SBUF (DVE copies, hh*64 row
            # placement), raw Z bounced through DRAM with a partition-
            # broadcast read, reciprocal on the broadcast [128, ICH] tile,
            # one aligned multiply. All bounces run while the bus is quiet
            # (collectives are deferred past attention).
            cu = cupool.tile([128, ICH], F32, tag="cu", name=f"cu{ch}")
            zst = zpool.tile([65, 2 * ICH], F32, tag="zst", name=f"zst{ch}")
            for hh in range(2):
                nc.vector.tensor_copy(cu[hh * 64:(hh + 1) * 64, :],
                                      cps[hh][0:64, :])
                nc.vector.tensor_copy(zst[64:65, hh * ICH:(hh + 1) * ICH],
                                      cps[hh][64:65, :])
            nc.sync.dma_start(
                bass.AP(zscr[b].ap().tensor, ch2 * ICH, [[S, 2], [1, ICH]]),
                zst[64:65, :])
            zbc = zbpool.tile([128, ICH], F32, tag="zbc", name=f"zbc{ch}")
            nc.sync.dma_start(
                zbc[:],
                bass.AP(zscr[b].ap().tensor, ch2 * ICH,
                        [[S, 2], [0, 64], [1, ICH]]))
            zbr = zbpool.tile([128, ICH], F32, tag="zbr", name=f"zbr{ch}")
            nc.vector.reciprocal_approx_fast(zbr[:], zbc[:])
            cn = cnpool.tile([128, ICH], BF16, tag="cn", name=f"cn{ch}")
            nc.vector.tensor_mul(cn[:], cu[:], zbr[:])
            # stage: dst core d gets this chunk's i-block d
            for dl in range(ICH // SEQ_PER_CORE):
                d = ch2 * (ICH // SEQ_PER_CORE) + dl
                nc.sync.dma_start(
                    a2a_in[b].ap()[d * 128:(d + 1) * 128, :],
                    cn[:, dl * SEQ_PER_CORE:(dl + 1) * SEQ_PER_CORE])

        def emit_cc(b):
            nc.gpsimd.collective_compute(
                "AllToAll", mybir.AluOpType.bypass,
                replica_groups=[list(range(N_CORES))],
                ins=[a2a_in[b].ap().opt()], outs=[a2a_out[b].ap().opt()])

        def emit_osrc(b):
            for s in range(N_CORES):
                nc.sync.dma_start(
                    osrc_sb[:, (b * N_CORES + s) * SEQ_PER_CORE:
                            (b * N_CORES + s + 1) * SEQ_PER_CORE],
                    a2a_out[b].ap()[s * 128:(s + 1) * 128, :])

        def emit_oproj(b):
            for ce in range(D // NMM):
                psums = [pools["op"].tile([128, NMM], F32, tag="op",
                                          name=f"op{b}_{ce}_{i}")
                         for i in range(2)]
                for s in range(N_CORES):
                    wo_t = wo_sb[:, (ce * N_CORES + s) * NMM:
                                 (ce * N_CORES + s + 1) * NMM]
                    for it in range(2):
                        nc.tensor.matmul(
                            psums[it][:],
                            osrc_sb[:, (b * N_CORES + s) * SEQ_PER_CORE +
                                    it * 128:
                                    (b * N_CORES + s) * SEQ_PER_CORE +
                                    (it + 1) * 128],
                            wo_t, start=(s == 0), stop=(s == N_CORES - 1))
                for it in range(2):
                    ot = outpool.tile([128, NMM], F32, tag="ot",
                                      name=f"ot{b}_{ce}_{it}")
                    nc.vector.tensor_add(ot[:], psums[it][:],
                                         bo_sb[:, ce * NMM:(ce + 1) * NMM])
                    nc.sync.dma_start(
                        out_d.ap()[b * SEQ_PER_CORE + it * 128:
                                   b * SEQ_PER_CORE + (it + 1) * 128,
                                   ce * NMM:(ce + 1) * NMM], ot[:])

        # All projections first (the x stream is chip-HBM-bound and PSUM
        # can't host proj and attention at once); attention follows with
        # per-chunk normalize/staging and the batch-0 collective hidden
        # under batch-1's attention.
        with tc.tile_pool(name="proj_psum", bufs=2, space="PSUM") as pp:
            pools["pp"] = pp
            emit_proj_half("k", 0)
            emit_proj_half("v", 0)
            emit_proj_half("q", 0)
            emit_proj_half("k", 1)
            emit_proj_half("v", 1)
            emit_proj_half("q", 1)
            emit_transposes(0)
            emit_transposes(1)

        with tc.tile_pool(name="spsum", bufs=2, space="PSUM") as sp, \
             tc.tile_pool(name="cpsum", bufs=2, space="PSUM") as cp:
            pools["sp"], pools["cp"] = sp, cp
            for ch in range(NCH):
                cps = emit_attention_chunk(ch)
                emit_norm_stage(ch, cps)
                if ch == 1:
                    emit_cc(0)
            emit_osrc(0)
            emit_cc(1)
            emit_osrc(1)

        with tc.tile_pool(name="opsum", bufs=4, space="PSUM") as op:
            pools["op"] = op
            emit_oproj(0)
            emit_oproj(1)


def shard_inputs(inputs):
    q, k, v = inputs["query"], inputs["key"], inputs["value"]
    xt = {}
    for t, x in (("q", q), ("k", k), ("v", v)):
        xt[t] = np.ascontiguousarray(
            np.asarray(x, np.float32).transpose(2, 1, 0).reshape(D, SB)
        ).astype(ml_dtypes.bfloat16)
    woT = np.ascontiguousarray(
        np.asarray(inputs["w_o"], np.float32).T).astype(ml_dtypes.bfloat16)
    bo_bc = np.ascontiguousarray(
        np.tile(np.asarray(inputs["b_o"], np.float32).reshape(1, D), (128, 1)))
    in_maps = []
    for c in range(N_CORES):
        m = {"woT": woT, "bo_bc": bo_bc}
        for t in "qkv":
            m[f"x{t}T"] = xt[t]
            w = np.asarray(inputs[f"w_{t}"], np.float32)
            bb = np.asarray(inputs[f"b_{t}"], np.float32)
            m[f"w{t}T"] = np.ascontiguousarray(
                w[c * 128:(c + 1) * 128, :].T).astype(ml_dtypes.bfloat16)
            m[f"b{t}"] = np.ascontiguousarray(
                bb[c * 128:(c + 1) * 128].reshape(128, 1))
        in_maps.append(m)
    return in_maps


def unshard(results):
    out = np.empty((S, B, D), np.float32)
    for c in range(N_CORES):
        o = results[c]["out"]          # [512, 1024], row r = b*256 + rr
        for b in range(B):
            out[c * SEQ_PER_CORE:(c + 1) * SEQ_PER_CORE, b, :] = \
                o[b * SEQ_PER_CORE:(b + 1) * SEQ_PER_CORE, :]
    return out


def run(inputs, trace=False, trace_cores=None):
    nc = build_program()
    in_maps = shard_inputs(inputs)
    res = run_bass_kernel_spmd(nc, in_maps, core_ids=list(range(N_CORES)),
                               trace=trace, trace_cores=trace_cores)
    return unshard(res.results), res


def kernel(**inputs):
    out, _ = run(inputs, trace=False)
    return out
